# revision 6
# baseline (speedup 1.0000x reference)
"""Euler-Maruyama SDE paths on Trainium2 (Bass/Tile, 8 NeuronCores).

Recurrence: Z[:, t] = Z[:, t-1] * (1 + r*dt + s*sqrt(dt)*W[:, t]), Z[:, 0] = Z0.

Memory-regime optimization: W is quantized host-side to fp8 (e3m4) and Z is
produced in fp16 (fp32 scan state, downcast on write), then upcast host-side.
Per-core HBM traffic drops from 134 MB (fp32 in/out) to 50 MB.

Compute optimization ("pair" design): the DVE scan instruction costs ~2.1
cycles/element regardless of dtype, so the time axis is pair-compressed:
  g_k  = m_{2k-1} * m_{2k}            (Pool-engine TT, fp32)
  Z_2k = scan over g (fp32 state)     (DVE scan over N/2 elements)
  Z_odd = Z_even_shifted * m_odd      (DVE fp16 TT, one multiply, no chain)
W rows are de-interleaved host-side ([odds | evens]) so every device access
is contiguous; the host re-interleaves the two output halves.

Sharding: batch (path) dim split evenly across the 8 cores (pure data
parallel); the time recurrence stays on-core; weights baked as immediates.
"""

import numpy as np

import concourse.bacc as bacc
import concourse.bass as bass
import concourse.mybir as mybir
import concourse.tile as tile
from concourse.bass_utils import run_bass_kernel_spmd

N_CORES = 8
B = 131072
NT = 1024  # time steps; output has NT+1 columns
NH = NT // 2  # 512 pairs
ROWS = B // N_CORES  # 16384 rows per core
P = 128  # SBUF partitions
RPP = 4  # rows per partition per tile
G = ROWS // (P * RPP)  # tiles per core

F32 = mybir.dt.float32
F16 = mybir.dt.float16
F8 = mybir.dt.float8e3


def _build_nc_pair(rows: int, nt: int, r: float, s: float, rpp: int,
                   w_bufs: int = 4, m_bufs: int = 3, g_bufs: int = 3,
                   mo_bufs: int = 3, oe_bufs: int = 4, zo_bufs: int = 4):
    """Pair-compressed per-core program.  W8 input is de-interleaved per row:
    cols [0:512] = W[:, 1::2] (odd t), cols [512:1024] = W[:, 2::2] (even t).
    Z output [rows, 1024]: cols [0:512] = Z odd t, cols [512:1024] = Z even t.
    """
    nh = nt // 2
    dt = np.float32(1.0 / nt)
    sdt = np.float32(np.sqrt(dt))
    scale = float(np.float32(s) * sdt)
    bias = float(np.float32(1.0) + np.float32(r) * dt)

    g = rows // (P * rpp)
    assert rows == P * rpp * g

    ident = mybir.ActivationFunctionType.Identity
    mult = mybir.AluOpType.mult
    add = mybir.AluOpType.add

    nc = bacc.Bacc("TRN2", target_bir_lowering=False, debug=False,
                   num_devices=N_CORES)
    W8 = nc.dram_tensor("W8", [rows, nt], F8, kind="ExternalInput").ap()
    Z0 = nc.dram_tensor("Z0", [rows], F32, kind="ExternalInput").ap()
    Z = nc.dram_tensor("Z", [rows, nt], F16, kind="ExternalOutput").ap()

    W_v = W8.rearrange("(p t j) c -> p t j c", p=P, t=g, j=rpp)
    Z_v = Z.rearrange("(p t j) c -> p t j c", p=P, t=g, j=rpp)
    Z0_v = Z0.rearrange("(p m) -> p m", p=P)  # col m = t*rpp + j

    GW = nh + 2  # pair-product row width: [0, g_1..g_512, 0]

    with tile.TileContext(nc) as tc:
        with (
            tc.tile_pool(name="z0", bufs=1) as z0_pool,
            tc.tile_pool(name="w", bufs=w_bufs) as w_pool,
            tc.tile_pool(name="m", bufs=m_bufs) as m_pool,
            tc.tile_pool(name="gp", bufs=g_bufs) as g_pool,
            tc.tile_pool(name="mo", bufs=mo_bufs) as mo_pool,
            tc.tile_pool(name="oe", bufs=oe_bufs) as oe_pool,
            tc.tile_pool(name="zo", bufs=zo_bufs) as zo_pool,
        ):
            z0_all = z0_pool.tile([P, rpp * g], F32)
            nc.sync.dma_start(z0_all[:], Z0_v[:])
            bias_t = z0_pool.tile([P, 1], F32, tag="bias")
            nc.vector.memset(bias_t[:], bias)
            # data1 tiles for the scan (parity double-buffer): zeros except
            # col 0 of each row = Z0 (reset injection); zero cols persist.
            n_at = 4
            ats = [z0_pool.tile([P, rpp, GW], F32, tag=f"at{q}",
                                name=f"at{q}")
                   for q in range(n_at)]
            for q in range(n_at):
                nc.gpsimd.memset(ats[q][:], 0.0)

            pending = []  # delayed out-DMAs: (t, zo_ap, oe_ap)

            def flush_outdma():
                for (tp, zo_p, oe_p) in pending:
                    nc.gpsimd.dma_start(Z_v[:, tp, :, 0:nh], zo_p)
                    nc.gpsimd.dma_start(Z_v[:, tp, :, nh:nt], oe_p)
                pending.clear()

            for t in range(g):
                at = ats[t % n_at]
                wt = w_pool.tile([P, rpp, nt], F8, tag="w")
                mt = m_pool.tile([P, rpp, nt], F32, tag="m")
                mo = mo_pool.tile([P, rpp, nh], F16, tag="mo")
                gt = g_pool.tile([P, rpp, GW], F32, tag="g")
                oe = oe_pool.tile([P, rpp, GW], F16, tag="oe")
                zo = zo_pool.tile([P, rpp, nh], F16, tag="zo")

                nc.sync.dma_start(wt[:], W_v[:, t])
                # fp8 -> fp32 affine: m for both halves
                nc.scalar.activation(mt[:], wt[:], ident,
                                     bias=bias_t[:], scale=scale)
                # fp8 -> fp16 affine: odd-t m for the reconstruct multiply
                nc.scalar.activation(mo[:], wt[:, :, 0:nh], ident,
                                     bias=bias_t[:], scale=scale)
                # Z0 reset values into data1 col 0 (ACT, ~free)
                nc.scalar.activation(at[:, :, 0],
                                     z0_all[:, t * rpp:(t + 1) * rpp], ident)
                # pair products on the Pool engine (fp32)
                nc.gpsimd.memset(gt[:, :, 0], 0.0)
                nc.gpsimd.memset(gt[:, :, GW - 1], 0.0)
                nc.gpsimd.tensor_tensor(out=gt[:, :, 1:nh + 1],
                                        in0=mt[:, :, 0:nh],
                                        in1=mt[:, :, nh:nt], op=mult)
                # previous tile's out-DMAs issue after this tile's pair TT
                # so their wait (on DVE) never starves the Pool queue
                flush_outdma()
                # chained scan over all rpp rows: state = g*state + reset
                # row layout [0|g..|0] -> emits [Z0, Z_2, ..., Z_1024, 0]
                nc.vector.tensor_tensor_scan(
                    out=oe[:].rearrange("p j c -> p (j c)"),
                    data0=gt[:].rearrange("p j c -> p (j c)"),
                    data1=at[:].rearrange("p j c -> p (j c)"),
                    initial=0.0, op0=mult, op1=add)
                # odd outputs: one fp16 multiply, no chain
                nc.vector.tensor_tensor(out=zo[:], in0=oe[:, :, 0:nh],
                                        in1=mo[:], op=mult)
                pending.append((t, zo[:], oe[:, :, 1:nh + 1]))
            flush_outdma()

    nc.compile()
    return nc


def _build_nc_scan(rows: int, nt: int, r: float, s: float, rpp: int):
    """Fallback: plain per-row scans (fp8 in, fp16 out), W8 = [rows, nt+1]
    natural layout, Z = [rows, nt+1]."""
    dt = np.float32(1.0 / nt)
    sdt = np.float32(np.sqrt(dt))
    scale = float(np.float32(s) * sdt)
    bias = float(np.float32(1.0) + np.float32(r) * dt)
    g = rows // (P * rpp)

    ident = mybir.ActivationFunctionType.Identity
    mult = mybir.AluOpType.mult
    bypass = mybir.AluOpType.bypass

    nc = bacc.Bacc("TRN2", target_bir_lowering=False, debug=False,
                   num_devices=N_CORES)
    W8 = nc.dram_tensor("W8", [rows, nt + 1], F8, kind="ExternalInput").ap()
    Z0 = nc.dram_tensor("Z0", [rows], F32, kind="ExternalInput").ap()
    Z = nc.dram_tensor("Z", [rows, nt + 1], F16, kind="ExternalOutput").ap()

    W_v = W8.rearrange("(p t j) c -> p t j c", p=P, t=g, j=rpp)
    Z_v = Z.rearrange("(p t j) c -> p t j c", p=P, t=g, j=rpp)
    Z0_v = Z0.rearrange("(p m) -> p m", p=P)

    with tile.TileContext(nc) as tc:
        with (
            tc.tile_pool(name="z0", bufs=1) as z0_pool,
            tc.tile_pool(name="w", bufs=4) as w_pool,
            tc.tile_pool(name="m", bufs=3) as m_pool,
            tc.tile_pool(name="o", bufs=4) as o_pool,
        ):
            z0_all = z0_pool.tile([P, rpp * g], F32)
            nc.sync.dma_start(z0_all[:], Z0_v[:])
            bias_t = z0_pool.tile([P, 1], F32, tag="bias")
            nc.vector.memset(bias_t[:], bias)

            for t in range(g):
                wt = w_pool.tile([P, rpp, nt + 1], F8, tag="w")
                mt = m_pool.tile([P, rpp, nt], F32, tag="m")
                ot = o_pool.tile([P, rpp, nt + 1], F16, tag="o")
                nc.sync.dma_start(wt[:], W_v[:, t])
                nc.scalar.activation(mt[:], wt[:, :, 1:], ident,
                                     bias=bias_t[:], scale=scale)
                nc.scalar.activation(
                    ot[:, :, 0], z0_all[:, t * rpp:(t + 1) * rpp], ident)
                for j in range(rpp):
                    nc.vector.tensor_tensor_scan(
                        out=ot[:, j, 1:], data0=mt[:, j, :],
                        data1=mt[:, j, :],
                        initial=z0_all[:, t * rpp + j: t * rpp + j + 1],
                        op0=mult, op1=bypass)
                nc.gpsimd.dma_start(Z_v[:, t], ot[:])

    nc.compile()
    return nc


_NC_CACHE: dict = {}


def _get_nc(r: float, s: float, design: str):
    key = (r, s, design)
    if key not in _NC_CACHE:
        build = _build_nc_pair if design == "pair" else _build_nc_scan
        _NC_CACHE[key] = build(ROWS, NT, r, s, RPP)
    return _NC_CACHE[key]


_JIT_CACHE: dict = {}


def _get_sharded_fn(nc):
    """jit(shard_map) callable for the per-core Bass program, inputs
    pre-placed on device so no H2D traffic overlaps kernel execution."""
    if id(nc) in _JIT_CACHE:
        return _JIT_CACHE[id(nc)]

    import jax
    from jax.sharding import Mesh, NamedSharding, PartitionSpec
    from jax.experimental.shard_map import shard_map

    from concourse import bass2jax
    from concourse.bass2jax import _bass_exec_p, partition_id_tensor

    bass2jax.install_neuronx_cc_hook()

    partition_name = (nc.partition_id_tensor.name
                      if nc.partition_id_tensor else None)
    in_names, out_names, out_avals = [], [], []
    for alloc in nc.m.functions[0].allocations:
        if not isinstance(alloc, mybir.MemoryLocationSet):
            continue
        name = alloc.memorylocations[0].name
        if alloc.kind == "ExternalInput":
            if name != partition_name:
                in_names.append(name)
        elif alloc.kind == "ExternalOutput":
            out_names.append(name)
            out_avals.append(jax.core.ShapedArray(
                tuple(alloc.tensor_shape), mybir.dt.np(alloc.dtype)))
    n_params = len(in_names)
    all_in_names = list(in_names) + list(out_names)
    if partition_name is not None:
        all_in_names.append(partition_name)

    def _body(*args):
        operands = list(args)
        if partition_name is not None:
            operands.append(partition_id_tensor())
        outs = _bass_exec_p.bind(
            *operands,
            out_avals=tuple(out_avals),
            in_names=tuple(all_in_names),
            out_names=tuple(out_names),
            lowering_input_output_aliases=(),
            sim_require_finite=True,
            sim_require_nnan=True,
            nc=nc,
        )
        return tuple(outs)

    devices = jax.devices()[:N_CORES]
    mesh = Mesh(np.asarray(devices), ("core",))
    sharding = NamedSharding(mesh, PartitionSpec("core"))
    n_outs = len(out_avals)
    donate = tuple(range(n_params, n_params + n_outs))
    sharded = jax.jit(
        shard_map(_body, mesh=mesh,
                  in_specs=(PartitionSpec("core"),) * (n_params + n_outs),
                  out_specs=(PartitionSpec("core"),) * n_outs,
                  check_rep=False),
        donate_argnums=donate, keep_unused=True,
    )
    zeros_fn = jax.jit(
        lambda: tuple(
            jax.numpy.zeros((N_CORES * a.shape[0], *a.shape[1:]), a.dtype)
            for a in out_avals),
        out_shardings=tuple(sharding for _ in out_avals),
    )
    entry = (sharded, zeros_fn, in_names, out_names, out_avals, sharding)
    _JIT_CACHE[id(nc)] = entry
    return entry


def _prep_w(W: np.ndarray, design: str) -> np.ndarray:
    """Host-side W preparation (jax cpu): quantize fp32 -> fp8 e3m4; for the
    pair design also de-interleave rows to [odds | evens]."""
    import jax
    import jax.numpy as jnp
    import ml_dtypes

    cpu = jax.devices("cpu")[0]
    with jax.default_device(cpu):
        if design == "pair":
            f = jax.jit(lambda x: jnp.concatenate(
                [x[:, 1::2], x[:, 2::2]], axis=1).astype(jnp.float8_e3m4))
        else:
            f = jax.jit(lambda x: x.astype(jnp.float8_e3m4))
        W8 = np.asarray(f(W))
    return W8.view(ml_dtypes.float8_e3m4)


def _post_z(Zdev: np.ndarray, Z0: np.ndarray, design: str) -> np.ndarray:
    """Upcast fp16 device output to fp32 and restore natural layout."""
    if design == "pair":
        Z = np.empty((Zdev.shape[0], NT + 1), np.float32)
        Z[:, 1::2] = Zdev[:, 0:NH]
        Z[:, 2::2] = Zdev[:, NH:NT]
    else:
        Z = Zdev.astype(np.float32)
    Z[:, 0] = Z0
    return Z


def run(Z0, W, Wf, Wg, profile_ctx=None, design: str = "pair"):
    import jax

    Z0 = np.ascontiguousarray(np.asarray(Z0, dtype=np.float32))
    W = np.asarray(W)
    r = float(np.asarray(Wf, dtype=np.float32)[0, 0])
    s = float(np.asarray(Wg, dtype=np.float32)[0, 0])
    nc = _get_nc(r, s, design)
    sharded, zeros_fn, in_names, out_names, out_avals, sharding = \
        _get_sharded_fn(nc)

    W8 = _prep_w(np.ascontiguousarray(W, dtype=np.float32), design)
    host_in = {"W8": W8, "Z0": Z0}
    dev_in = [jax.device_put(host_in[n], sharding) for n in in_names]
    dev_zeros = list(zeros_fn())
    jax.block_until_ready(dev_in + dev_zeros)

    if profile_ctx is not None:
        with profile_ctx:
            outs = jax.block_until_ready(sharded(*dev_in, *dev_zeros))
    else:
        outs = jax.block_until_ready(sharded(*dev_in, *dev_zeros))

    out_map = dict(zip(out_names, outs))
    Z = _post_z(np.asarray(out_map["Z"]), Z0, design)
    return (Z, W), nc


def _run_fallback(Z0, W, Wf, Wg, design: str = "pair"):
    """Stock dispatch via run_bass_kernel_spmd."""
    Z0 = np.ascontiguousarray(np.asarray(Z0, dtype=np.float32))
    W = np.asarray(W)
    r = float(np.asarray(Wf, dtype=np.float32)[0, 0])
    s = float(np.asarray(Wg, dtype=np.float32)[0, 0])
    nc = _get_nc(r, s, design)
    W8 = _prep_w(np.ascontiguousarray(W, dtype=np.float32), design)
    in_maps = [
        {"W8": W8[c * ROWS:(c + 1) * ROWS],
         "Z0": Z0[c * ROWS:(c + 1) * ROWS]}
        for c in range(N_CORES)
    ]
    res = run_bass_kernel_spmd(nc, in_maps, list(range(N_CORES)))
    Zdev = np.concatenate(
        [np.asarray(res.results[c]["Z"]) for c in range(N_CORES)], axis=0)
    return _post_z(Zdev, Z0, design), W


def kernel(Z0, W, Wf, Wg):
    for design in ("pair", "scan"):
        try:
            (Z, W_out), _ = run(Z0, W, Wf, Wg, design=design)
            return Z, W_out
        except Exception:
            try:
                return _run_fallback(Z0, W, Wf, Wg, design=design)
            except Exception:
                continue
    raise RuntimeError("all kernel designs failed")


# revision 7
# speedup vs baseline: 1.0226x; 1.0226x over previous
"""Euler-Maruyama SDE paths on Trainium2 (Bass/Tile, 8 NeuronCores).

Recurrence: Z[:, t] = Z[:, t-1] * (1 + r*dt + s*sqrt(dt)*W[:, t]), Z[:, 0] = Z0.

Memory-regime optimization: W is quantized host-side to fp8 (e3m4) and Z is
produced in fp16 (fp32 scan state, downcast on write), then upcast host-side.
Per-core HBM traffic drops from 134 MB (fp32 in/out) to 50 MB.

Compute optimization ("pair" design): the DVE scan instruction costs ~2.1
cycles/element regardless of dtype, so the time axis is pair-compressed:
  g_k  = m_{2k-1} * m_{2k}            (Pool-engine TT, fp32)
  Z_2k = scan over g (fp32 state)     (DVE scan over N/2 elements)
  Z_odd = Z_even_shifted * m_odd      (DVE fp16 TT, one multiply, no chain)
W rows are de-interleaved host-side ([odds | evens]) so every device access
is contiguous; the host re-interleaves the two output halves.

Sharding: batch (path) dim split evenly across the 8 cores (pure data
parallel); the time recurrence stays on-core; weights baked as immediates.
"""

import numpy as np

import concourse.bacc as bacc
import concourse.bass as bass
import concourse.mybir as mybir
import concourse.tile as tile
from concourse.bass_utils import run_bass_kernel_spmd

N_CORES = 8
B = 131072
NT = 1024  # time steps; output has NT+1 columns
NH = NT // 2  # 512 pairs
ROWS = B // N_CORES  # 16384 rows per core
P = 128  # SBUF partitions
RPP = 4  # rows per partition per tile
G = ROWS // (P * RPP)  # tiles per core

F32 = mybir.dt.float32
F16 = mybir.dt.float16
F8 = mybir.dt.float8e3


def _build_nc_pair(rows: int, nt: int, r: float, s: float, rpp: int,
                   w_bufs: int = 5, m_bufs: int = 3, g_bufs: int = 4,
                   mo_bufs: int = 3, oe_bufs: int = 4, zo_bufs: int = 4):
    """Pair-compressed per-core program.  W8 input is de-interleaved per row:
    cols [0:512] = W[:, 1::2] (odd t), cols [512:1024] = W[:, 2::2] (even t).
    Z output [rows, 1024]: cols [0:512] = Z odd t, cols [512:1024] = Z even t.
    """
    nh = nt // 2
    dt = np.float32(1.0 / nt)
    sdt = np.float32(np.sqrt(dt))
    scale = float(np.float32(s) * sdt)
    bias = float(np.float32(1.0) + np.float32(r) * dt)

    g = rows // (P * rpp)
    assert rows == P * rpp * g

    ident = mybir.ActivationFunctionType.Identity
    mult = mybir.AluOpType.mult
    add = mybir.AluOpType.add

    nc = bacc.Bacc("TRN2", target_bir_lowering=False, debug=False,
                   num_devices=N_CORES)
    W8 = nc.dram_tensor("W8", [rows, nt], F8, kind="ExternalInput").ap()
    Z0 = nc.dram_tensor("Z0", [rows], F32, kind="ExternalInput").ap()
    Z = nc.dram_tensor("Z", [rows, nt], F16, kind="ExternalOutput").ap()

    W_v = W8.rearrange("(p t j) c -> p t j c", p=P, t=g, j=rpp)
    Z_v = Z.rearrange("(p t j) c -> p t j c", p=P, t=g, j=rpp)
    Z0_v = Z0.rearrange("(p m) -> p m", p=P)  # col m = t*rpp + j

    GW = nh + 2  # pair-product row width: [0, g_1..g_512, 0]

    with tile.TileContext(nc) as tc:
        with (
            tc.tile_pool(name="z0", bufs=1) as z0_pool,
            tc.tile_pool(name="w", bufs=w_bufs) as w_pool,
            tc.tile_pool(name="m", bufs=m_bufs) as m_pool,
            tc.tile_pool(name="gp", bufs=g_bufs) as g_pool,
            tc.tile_pool(name="mo", bufs=mo_bufs) as mo_pool,
            tc.tile_pool(name="oe", bufs=oe_bufs) as oe_pool,
            tc.tile_pool(name="zo", bufs=zo_bufs) as zo_pool,
        ):
            z0_all = z0_pool.tile([P, rpp * g], F32)
            nc.sync.dma_start(z0_all[:], Z0_v[:])
            bias_t = z0_pool.tile([P, 1], F32, tag="bias")
            nc.vector.memset(bias_t[:], bias)
            # data1 tiles for the scan (parity double-buffer): zeros except
            # col 0 of each row = Z0 (reset injection); zero cols persist.
            n_at = 4
            ats = [z0_pool.tile([P, rpp, GW], F32, tag=f"at{q}",
                                name=f"at{q}")
                   for q in range(n_at)]
            for q in range(n_at):
                nc.gpsimd.memset(ats[q][:], 0.0)

            pending = []  # delayed out-DMAs: (t, zo_ap, oe_ap)

            def flush_outdma(keep: int = 0):
                while len(pending) > keep:
                    tp, zo_p, oe_p = pending.pop(0)
                    nc.sync.dma_start(Z_v[:, tp, :, 0:nh], zo_p)
                    nc.sync.dma_start(Z_v[:, tp, :, nh:nt], oe_p)

            for t in range(g):
                at = ats[t % n_at]
                wt = w_pool.tile([P, rpp, nt], F8, tag="w")
                mt = m_pool.tile([P, rpp, nt], F32, tag="m")
                mo = mo_pool.tile([P, rpp, nh], F16, tag="mo")
                gt = g_pool.tile([P, rpp, GW], F32, tag="g")
                oe = oe_pool.tile([P, rpp, GW], F16, tag="oe")
                zo = zo_pool.tile([P, rpp, nh], F16, tag="zo")

                nc.sync.dma_start(wt[:], W_v[:, t])
                # out-DMAs (2 tiles old: recon is long done, so the issue
                # never blocks, keeping in-DMA prefetch flowing)
                flush_outdma(keep=2)
                # fp8 -> fp32 affine: m for both halves
                nc.scalar.activation(mt[:], wt[:], ident,
                                     bias=bias_t[:], scale=scale)
                # fp8 -> fp16 affine: odd-t m for the reconstruct multiply
                nc.scalar.activation(mo[:], wt[:, :, 0:nh], ident,
                                     bias=bias_t[:], scale=scale)
                # Z0 reset values into data1 col 0 (ACT, ~free)
                nc.scalar.activation(at[:, :, 0],
                                     z0_all[:, t * rpp:(t + 1) * rpp], ident)
                # pair products on the Pool engine (fp32)
                nc.gpsimd.memset(gt[:, :, 0], 0.0)
                nc.gpsimd.memset(gt[:, :, GW - 1], 0.0)
                nc.gpsimd.tensor_tensor(out=gt[:, :, 1:nh + 1],
                                        in0=mt[:, :, 0:nh],
                                        in1=mt[:, :, nh:nt], op=mult)
                # chained scan over all rpp rows: state = g*state + reset
                # row layout [0|g..|0] -> emits [Z0, Z_2, ..., Z_1024, 0]
                nc.vector.tensor_tensor_scan(
                    out=oe[:].rearrange("p j c -> p (j c)"),
                    data0=gt[:].rearrange("p j c -> p (j c)"),
                    data1=at[:].rearrange("p j c -> p (j c)"),
                    initial=0.0, op0=mult, op1=add)
                # odd outputs: one fp16 multiply, no chain
                nc.vector.tensor_tensor(out=zo[:], in0=oe[:, :, 0:nh],
                                        in1=mo[:], op=mult)
                pending.append((t, zo[:], oe[:, :, 1:nh + 1]))
            flush_outdma()

    nc.compile()
    return nc


def _build_nc_scan(rows: int, nt: int, r: float, s: float, rpp: int):
    """Fallback: plain per-row scans (fp8 in, fp16 out), W8 = [rows, nt+1]
    natural layout, Z = [rows, nt+1]."""
    dt = np.float32(1.0 / nt)
    sdt = np.float32(np.sqrt(dt))
    scale = float(np.float32(s) * sdt)
    bias = float(np.float32(1.0) + np.float32(r) * dt)
    g = rows // (P * rpp)

    ident = mybir.ActivationFunctionType.Identity
    mult = mybir.AluOpType.mult
    bypass = mybir.AluOpType.bypass

    nc = bacc.Bacc("TRN2", target_bir_lowering=False, debug=False,
                   num_devices=N_CORES)
    W8 = nc.dram_tensor("W8", [rows, nt + 1], F8, kind="ExternalInput").ap()
    Z0 = nc.dram_tensor("Z0", [rows], F32, kind="ExternalInput").ap()
    Z = nc.dram_tensor("Z", [rows, nt + 1], F16, kind="ExternalOutput").ap()

    W_v = W8.rearrange("(p t j) c -> p t j c", p=P, t=g, j=rpp)
    Z_v = Z.rearrange("(p t j) c -> p t j c", p=P, t=g, j=rpp)
    Z0_v = Z0.rearrange("(p m) -> p m", p=P)

    with tile.TileContext(nc) as tc:
        with (
            tc.tile_pool(name="z0", bufs=1) as z0_pool,
            tc.tile_pool(name="w", bufs=4) as w_pool,
            tc.tile_pool(name="m", bufs=3) as m_pool,
            tc.tile_pool(name="o", bufs=4) as o_pool,
        ):
            z0_all = z0_pool.tile([P, rpp * g], F32)
            nc.sync.dma_start(z0_all[:], Z0_v[:])
            bias_t = z0_pool.tile([P, 1], F32, tag="bias")
            nc.vector.memset(bias_t[:], bias)

            for t in range(g):
                wt = w_pool.tile([P, rpp, nt + 1], F8, tag="w")
                mt = m_pool.tile([P, rpp, nt], F32, tag="m")
                ot = o_pool.tile([P, rpp, nt + 1], F16, tag="o")
                nc.sync.dma_start(wt[:], W_v[:, t])
                nc.scalar.activation(mt[:], wt[:, :, 1:], ident,
                                     bias=bias_t[:], scale=scale)
                nc.scalar.activation(
                    ot[:, :, 0], z0_all[:, t * rpp:(t + 1) * rpp], ident)
                for j in range(rpp):
                    nc.vector.tensor_tensor_scan(
                        out=ot[:, j, 1:], data0=mt[:, j, :],
                        data1=mt[:, j, :],
                        initial=z0_all[:, t * rpp + j: t * rpp + j + 1],
                        op0=mult, op1=bypass)
                nc.gpsimd.dma_start(Z_v[:, t], ot[:])

    nc.compile()
    return nc


_NC_CACHE: dict = {}


def _get_nc(r: float, s: float, design: str):
    key = (r, s, design)
    if key not in _NC_CACHE:
        build = _build_nc_pair if design == "pair" else _build_nc_scan
        _NC_CACHE[key] = build(ROWS, NT, r, s, RPP)
    return _NC_CACHE[key]


_JIT_CACHE: dict = {}


def _get_sharded_fn(nc):
    """jit(shard_map) callable for the per-core Bass program, inputs
    pre-placed on device so no H2D traffic overlaps kernel execution."""
    if id(nc) in _JIT_CACHE:
        return _JIT_CACHE[id(nc)]

    import jax
    from jax.sharding import Mesh, NamedSharding, PartitionSpec
    from jax.experimental.shard_map import shard_map

    from concourse import bass2jax
    from concourse.bass2jax import _bass_exec_p, partition_id_tensor

    bass2jax.install_neuronx_cc_hook()

    partition_name = (nc.partition_id_tensor.name
                      if nc.partition_id_tensor else None)
    in_names, out_names, out_avals = [], [], []
    for alloc in nc.m.functions[0].allocations:
        if not isinstance(alloc, mybir.MemoryLocationSet):
            continue
        name = alloc.memorylocations[0].name
        if alloc.kind == "ExternalInput":
            if name != partition_name:
                in_names.append(name)
        elif alloc.kind == "ExternalOutput":
            out_names.append(name)
            out_avals.append(jax.core.ShapedArray(
                tuple(alloc.tensor_shape), mybir.dt.np(alloc.dtype)))
    n_params = len(in_names)
    all_in_names = list(in_names) + list(out_names)
    if partition_name is not None:
        all_in_names.append(partition_name)

    def _body(*args):
        operands = list(args)
        if partition_name is not None:
            operands.append(partition_id_tensor())
        outs = _bass_exec_p.bind(
            *operands,
            out_avals=tuple(out_avals),
            in_names=tuple(all_in_names),
            out_names=tuple(out_names),
            lowering_input_output_aliases=(),
            sim_require_finite=True,
            sim_require_nnan=True,
            nc=nc,
        )
        return tuple(outs)

    devices = jax.devices()[:N_CORES]
    mesh = Mesh(np.asarray(devices), ("core",))
    sharding = NamedSharding(mesh, PartitionSpec("core"))
    n_outs = len(out_avals)
    donate = tuple(range(n_params, n_params + n_outs))
    sharded = jax.jit(
        shard_map(_body, mesh=mesh,
                  in_specs=(PartitionSpec("core"),) * (n_params + n_outs),
                  out_specs=(PartitionSpec("core"),) * n_outs,
                  check_rep=False),
        donate_argnums=donate, keep_unused=True,
    )
    zeros_fn = jax.jit(
        lambda: tuple(
            jax.numpy.zeros((N_CORES * a.shape[0], *a.shape[1:]), a.dtype)
            for a in out_avals),
        out_shardings=tuple(sharding for _ in out_avals),
    )
    entry = (sharded, zeros_fn, in_names, out_names, out_avals, sharding)
    _JIT_CACHE[id(nc)] = entry
    return entry


def _prep_w(W: np.ndarray, design: str) -> np.ndarray:
    """Host-side W preparation (jax cpu): quantize fp32 -> fp8 e3m4; for the
    pair design also de-interleave rows to [odds | evens]."""
    import jax
    import jax.numpy as jnp
    import ml_dtypes

    cpu = jax.devices("cpu")[0]
    with jax.default_device(cpu):
        if design == "pair":
            f = jax.jit(lambda x: jnp.concatenate(
                [x[:, 1::2], x[:, 2::2]], axis=1).astype(jnp.float8_e3m4))
        else:
            f = jax.jit(lambda x: x.astype(jnp.float8_e3m4))
        W8 = np.asarray(f(W))
    return W8.view(ml_dtypes.float8_e3m4)


def _post_z(Zdev: np.ndarray, Z0: np.ndarray, design: str) -> np.ndarray:
    """Upcast fp16 device output to fp32 and restore natural layout."""
    if design == "pair":
        Z = np.empty((Zdev.shape[0], NT + 1), np.float32)
        Z[:, 1::2] = Zdev[:, 0:NH]
        Z[:, 2::2] = Zdev[:, NH:NT]
    else:
        Z = Zdev.astype(np.float32)
    Z[:, 0] = Z0
    return Z


def run(Z0, W, Wf, Wg, profile_ctx=None, design: str = "pair"):
    import jax

    Z0 = np.ascontiguousarray(np.asarray(Z0, dtype=np.float32))
    W = np.asarray(W)
    r = float(np.asarray(Wf, dtype=np.float32)[0, 0])
    s = float(np.asarray(Wg, dtype=np.float32)[0, 0])
    nc = _get_nc(r, s, design)
    sharded, zeros_fn, in_names, out_names, out_avals, sharding = \
        _get_sharded_fn(nc)

    W8 = _prep_w(np.ascontiguousarray(W, dtype=np.float32), design)
    host_in = {"W8": W8, "Z0": Z0}
    dev_in = [jax.device_put(host_in[n], sharding) for n in in_names]
    dev_zeros = list(zeros_fn())
    jax.block_until_ready(dev_in + dev_zeros)

    if profile_ctx is not None:
        with profile_ctx:
            outs = jax.block_until_ready(sharded(*dev_in, *dev_zeros))
    else:
        outs = jax.block_until_ready(sharded(*dev_in, *dev_zeros))

    out_map = dict(zip(out_names, outs))
    Z = _post_z(np.asarray(out_map["Z"]), Z0, design)
    return (Z, W), nc


def _run_fallback(Z0, W, Wf, Wg, design: str = "pair"):
    """Stock dispatch via run_bass_kernel_spmd."""
    Z0 = np.ascontiguousarray(np.asarray(Z0, dtype=np.float32))
    W = np.asarray(W)
    r = float(np.asarray(Wf, dtype=np.float32)[0, 0])
    s = float(np.asarray(Wg, dtype=np.float32)[0, 0])
    nc = _get_nc(r, s, design)
    W8 = _prep_w(np.ascontiguousarray(W, dtype=np.float32), design)
    in_maps = [
        {"W8": W8[c * ROWS:(c + 1) * ROWS],
         "Z0": Z0[c * ROWS:(c + 1) * ROWS]}
        for c in range(N_CORES)
    ]
    res = run_bass_kernel_spmd(nc, in_maps, list(range(N_CORES)))
    Zdev = np.concatenate(
        [np.asarray(res.results[c]["Z"]) for c in range(N_CORES)], axis=0)
    return _post_z(Zdev, Z0, design), W


def kernel(Z0, W, Wf, Wg):
    for design in ("pair", "scan"):
        try:
            (Z, W_out), _ = run(Z0, W, Wf, Wg, design=design)
            return Z, W_out
        except Exception:
            try:
                return _run_fallback(Z0, W, Wf, Wg, design=design)
            except Exception:
                continue
    raise RuntimeError("all kernel designs failed")
